# revision 8
# baseline (speedup 1.0000x reference)
"""Trainium2 Bass kernel for nn_CapsuleLayer (capsule conv + 3-iter routing).

Reference (per batch image, C=128, H=W=32, K=3, pad=1):
  priors[h,w,t,nc] = sum_c x_pad[c,h+i,w+j] * W[t,c,nc] + b[t,nc]
  o = mean_t priors
  3x: d2 = sum_cch (o - p_t)^2 ; cw = rsqrt(d2 + 1e-4)
      cw = cw / sum_t cw ; o = sum_t cw_t p_t
  out[nc,h,w] = o

Sharding: data-parallel over batch; 8 cores, one image each; weight/bias
replicated; no collectives.

Implementation notes (m-space routing):
- Work in deviations m_t = p_t - o. Since sum_t alpha_t = 1:
    w = sum_t alpha_t m_t;  o' = o + w;  m'_t = m_t - w
  and d2_t = ||m_t||^2 directly (no <o,p> pass, no ||p||^2 pass, no dist
  assembly). Per iteration the DVE does only: d2 halving-reduce, the
  alpha chain, q = alpha*m, w halving-reduce, m-update.
- The square for d2 runs on the ACT engine (DVE is the bottleneck).
- bf16 on-chip, fp32 PSUM for matmuls; layout [128pos, grp, tap9, cch16,
  cap32] keeps every big DVE op in 2x_1P mode (innermost step-1 cap runs).
- The (cch,cap) permuted channel order is undone for free inside the
  output DMA's address pattern (no un-permute pass).
"""

import numpy as np

C = 128
H = W = 32
B = 8
KK = 9
NCAPS = 32
CCH = 16
NC = NCAPS * CCH  # 512
NIT = 3
NPOS = H * W
CHUNK = 128
GRP = 2  # position-chunks per routing pass
NGRP = NPOS // (CHUNK * GRP)
PADW = 34

_cache = {}


def _build(with_bias: bool):
    import concourse.bass as bass
    import concourse.tile as tile
    from concourse import bacc, mybir
    from concourse.masks import make_identity

    f32 = mybir.dt.float32
    bf16 = mybir.dt.bfloat16
    AF = mybir.ActivationFunctionType

    nc = bacc.Bacc()
    x_d = nc.dram_tensor("x", [C, H, W], f32, kind="ExternalInput")
    w_d = nc.dram_tensor("w", [KK, C, NC], f32, kind="ExternalInput")
    b_d = nc.dram_tensor("b", [KK, NC], f32, kind="ExternalInput")
    out_d = nc.dram_tensor("out", [NC, NPOS], f32, kind="ExternalOutput")

    with tile.TileContext(nc) as tc:
        with (
            tc.tile_pool(name="singles", bufs=1) as singles,
            tc.tile_pool(name="priors", bufs=2) as priors_pool,
            tc.tile_pool(name="m", bufs=2) as m_pool,
            tc.tile_pool(name="big", bufs=1) as big_pool,
            tc.tile_pool(name="half", bufs=1) as half_pool,
            tc.tile_pool(name="wh", bufs=1) as wh_pool,
            tc.tile_pool(name="w", bufs=2) as w_pool,
            tc.tile_pool(name="oa", bufs=2) as oa_pool,
            tc.tile_pool(name="small", bufs=2) as small_pool,
            tc.tile_pool(name="ostage", bufs=2) as ostage_pool,
            tc.tile_pool(name="pp", bufs=4, space="PSUM") as pp,
            tc.tile_pool(name="mp", bufs=2, space="PSUM") as mp,
            tc.tile_pool(name="tpp", bufs=2, space="PSUM") as tpp,
        ):
            # ---- stage inputs: pad + cast to bf16 via gpsimd cast-DMA ----
            xpad = singles.tile([C, PADW * PADW], bf16)
            xpad_v = xpad[:].rearrange("p (h w) -> p h w", h=PADW)
            # zero only the border so the interior cast-DMA needn't wait on
            # a full-tile memset
            nc.gpsimd.memset(xpad_v[:, 0], 0.0)
            nc.gpsimd.memset(xpad_v[:, PADW - 1], 0.0)
            nc.gpsimd.memset(xpad_v[:, 1 : PADW - 1, 0], 0.0)
            nc.gpsimd.memset(xpad_v[:, 1 : PADW - 1, PADW - 1], 0.0)
            nc.gpsimd.dma_start(out=xpad_v[:, 1 : H + 1, 1 : W + 1], in_=x_d[:])

            # column-shifted padded images: xs[j][c, r*32+w] = xpad[c, r, w+j]
            # -> the (i,j)-tap patch for rows h0.. is the CONTIGUOUS slice
            #    xs[j][:, (h0+i)*32 : (h0+i)*32+128]
            xs = []
            for j in range(3):
                xj = singles.tile([C, PADW * W], bf16, tag=f"xs{j}")
                nc.sync.dma_start(
                    out=xj[:].rearrange("p (r w) -> p r w", r=PADW),
                    in_=xpad_v[:, :, j : j + W],
                )
                xs.append(xj)

            # W: natural-order bf16 load (cast in DMA), then per-tap ACT
            # permute (cap,cch)->(cch,cap) so the matmul rhs is contiguous
            wsb = []
            for t in range(KK):
                wt = singles.tile([C, NC], bf16, tag=f"wsb{t}")
                nc.gpsimd.dma_start(out=wt[:], in_=w_d[t])
                wp_t = singles.tile([C, CCH, NCAPS], bf16, tag=f"wsbp{t}")
                nc.scalar.copy(
                    out=wp_t[:],
                    in_=wt[:].rearrange("p (cap cch) -> p cch cap", cch=CCH),
                )
                wsb.append(wp_t)

            ident = singles.tile([128, 128], bf16)
            make_identity(nc, ident[:])

            eps = singles.tile([128, 1], f32)
            nc.gpsimd.memset(eps, 1e-4)

            if with_bias:
                braw = singles.tile([1, KK, NC], bf16)
                nc.gpsimd.dma_start(out=braw[:], in_=b_d[:].unsqueeze(0))
                bsb = singles.tile([1, KK, CCH, NCAPS], bf16)
                nc.scalar.copy(
                    out=bsb[:],
                    in_=braw[:].rearrange("p t (cap cch) -> p t cch cap", cch=CCH),
                )
                ones = singles.tile([1, CHUNK], bf16)
                nc.gpsimd.memset(ones, 1.0)

            for g in range(NGRP):
                # ---- priors + their tap-sum via PE ----
                priors = priors_pool.tile([128, GRP, KK, CCH, NCAPS], bf16)
                OA = oa_pool.tile([128, GRP, NC], bf16)  # running output o
                for cc in range(GRP):
                    ch = GRP * g + cc
                    om = mp.tile([128, NC], f32)  # sum_t priors (fp32)
                    for t in range(KK):
                        i, j = divmod(t, 3)
                        ps = pp.tile([128, NC], f32)
                        lhsT = xs[j][:, 128 * ch + 32 * i : 128 * ch + 32 * i + 128]
                        rhs = wsb[t][:].rearrange("p a b -> p (a b)")
                        if with_bias:
                            nc.tensor.matmul(
                                ps[:], lhsT, rhs, start=True, stop=False
                            )
                            brhs = bsb[:, t].rearrange("p a b -> p (a b)")
                            nc.tensor.matmul(
                                ps[:], ones[:], brhs, start=False, stop=True
                            )
                        else:
                            nc.tensor.matmul(ps[:], lhsT, rhs, start=True, stop=True)
                        nc.tensor.matmul(
                            om[:], lhsT, rhs, start=(t == 0), stop=(t == KK - 1)
                        )
                        if with_bias:
                            nc.tensor.matmul(
                                om[:], ones[:], brhs, start=False, stop=False,
                                skip_group_check=True,
                            )
                        if g == 0 and cc == 0:
                            # DVE is idle during the prologue: help drain the
                            # first chunk's PSUM so priors land sooner
                            nc.vector.tensor_copy(
                                out=priors[:, cc, t],
                                in_=ps[:].rearrange("p (a b) -> p a b", a=CCH),
                            )
                        else:
                            nc.scalar.copy(
                                out=priors[:, cc, t],
                                in_=ps[:].rearrange("p (a b) -> p a b", a=CCH),
                            )
                    # OA = o0 = mean_t priors
                    nc.scalar.activation(
                        out=OA[:, cc], in_=om[:], func=AF.Copy, scale=1.0 / KK
                    )

                # m0 = priors - o0
                m = m_pool.tile([128, GRP, KK, CCH, NCAPS], bf16)
                ob = (
                    OA[:]
                    .rearrange("p c (a b) -> p c a b", a=CCH)
                    .unsqueeze(2)
                    .broadcast_to((128, GRP, KK, CCH, NCAPS))
                )
                nc.vector.tensor_sub(m[:], priors[:], ob)

                for it in range(NIT):
                    last = it == NIT - 1
                    # d2_t = ||m_t||^2: ACT square + cch halving reduce
                    sq = big_pool.tile([128, GRP, KK, CCH, NCAPS], bf16, tag="big")
                    nc.scalar.activation(out=sq[:], in_=m[:], func=AF.Square)
                    h = half_pool.tile([128, GRP, KK, 8, NCAPS], bf16, tag="h")
                    nc.vector.tensor_add(
                        h[:], sq[:, :, :, 0:8], sq[:, :, :, 8:16]
                    )
                    nc.vector.tensor_add(
                        h[:, :, :, 0:4], h[:, :, :, 0:4], h[:, :, :, 4:8]
                    )
                    nc.vector.tensor_add(
                        h[:, :, :, 0:2], h[:, :, :, 0:2], h[:, :, :, 2:4]
                    )
                    d2 = small_pool.tile([128, GRP, KK, NCAPS], bf16, tag="d2")
                    nc.vector.tensor_add(d2[:], h[:, :, :, 0], h[:, :, :, 1])

                    # cwu = rsqrt(d2 + eps) on ACT (single table set)
                    cwu = small_pool.tile([128, GRP, KK, NCAPS], bf16, tag="cwu")
                    nc.scalar.activation(
                        out=cwu[:], in_=d2[:], func=AF.Abs_reciprocal_sqrt,
                        bias=eps[:],
                    )
                    # Z = sum_t cwu (halving over taps, final add in fp32)
                    zh = small_pool.tile([128, GRP, 4, NCAPS], bf16, tag="zh")
                    nc.vector.tensor_add(
                        zh[:], cwu[:, :, 0:4], cwu[:, :, 4:8]
                    )
                    nc.vector.tensor_add(
                        zh[:, :, 0:2], zh[:, :, 0:2], zh[:, :, 2:4]
                    )
                    nc.vector.tensor_add(zh[:, :, 0], zh[:, :, 0], zh[:, :, 1])
                    z = small_pool.tile([128, GRP, NCAPS], f32, tag="z")
                    nc.vector.tensor_add(z[:], zh[:, :, 0], cwu[:, :, 8])
                    rs = small_pool.tile([128, GRP, NCAPS], f32, tag="rs")
                    nc.vector.reciprocal_approx_fast(rs[:], z[:])
                    rsb = small_pool.tile([128, GRP, NCAPS], bf16, tag="rsb")
                    nc.vector.tensor_scalar_mul(rsb[:], rs[:], 1.0)
                    alpha = small_pool.tile([128, GRP, KK, NCAPS], bf16, tag="al")
                    nc.vector.tensor_mul(
                        alpha[:],
                        cwu[:],
                        rsb[:].unsqueeze(2).broadcast_to((128, GRP, KK, NCAPS)),
                    )

                    # q = alpha * m ; w = sum_t q (tap halving)
                    q = big_pool.tile([128, GRP, KK, CCH, NCAPS], bf16, tag="big")
                    ab = alpha[:].unsqueeze(3).broadcast_to(
                        (128, GRP, KK, CCH, NCAPS)
                    )
                    nc.vector.tensor_mul(q[:], m[:], ab)
                    qv = q[:].rearrange("p c t a b -> p c t (a b)")
                    wh = wh_pool.tile([128, GRP, 4, NC], bf16, tag="wh")
                    nc.vector.tensor_add(wh[:], qv[:, :, 0:4], qv[:, :, 4:8])
                    nc.vector.tensor_add(
                        wh[:, :, 0:2], wh[:, :, 0:2], wh[:, :, 2:4]
                    )
                    nc.vector.tensor_add(wh[:, :, 0], wh[:, :, 0], wh[:, :, 1])

                    if not last:
                        w = w_pool.tile([128, GRP, NC], bf16)
                        nc.vector.tensor_add(w[:], wh[:, :, 0], qv[:, :, 8])
                        nc.vector.tensor_add(OA[:], OA[:], w[:])
                        m2 = m_pool.tile([128, GRP, KK, CCH, NCAPS], bf16)
                        wb = (
                            w[:]
                            .rearrange("p c (a b) -> p c a b", a=CCH)
                            .unsqueeze(2)
                            .broadcast_to((128, GRP, KK, CCH, NCAPS))
                        )
                        nc.vector.tensor_sub(m2[:], m[:], wb)
                        m = m2
                        continue

                    # last iteration: per sub-chunk output assembly so the
                    # transposes overlap the other sub-chunk's tail
                    for cc in range(GRP):
                        ch = GRP * g + cc
                        wfin = w_pool.tile([128, NC], bf16, tag="wfin")
                        nc.vector.tensor_add(
                            wfin[:], wh[:, cc, 0], qv[:, cc, 8]
                        )
                        o3 = w_pool.tile([128, NC], bf16, tag="o3")
                        nc.vector.tensor_add(o3[:], OA[:, cc], wfin[:])
                        ot = ostage_pool.tile([128, 4, 128], f32, tag="ostage")
                        for blk in range(4):
                            tp = tpp.tile([128, 128], bf16)
                            nc.tensor.transpose(
                                tp[:],
                                o3[:, 128 * blk : 128 * (blk + 1)],
                                ident[:],
                            )
                            nc.scalar.copy(out=ot[:, blk], in_=tp[:])
                        # o3 is (cch,cap)-ordered; out_d wants nc=cap*16+cch.
                        # blk holds cch 4*blk..4*blk+3, partition = (c4,cap).
                        nc.sync.dma_start(
                            out=out_d[:, 128 * ch : 128 * (ch + 1)].rearrange(
                                "(cap blk c4) q -> c4 cap blk q", cap=NCAPS,
                                blk=4,
                            ),
                            in_=ot[:],
                        )
    nc.compile()
    return nc


def _get_nc(with_bias: bool):
    key = ("nc", with_bias)
    if key not in _cache:
        _cache[key] = _build(with_bias)
    return _cache[key]


def kernel(input, weight, bias, _trace=False):
    from concourse.bass_utils import run_bass_kernel_spmd

    input = np.ascontiguousarray(np.asarray(input, dtype=np.float32))
    w = np.ascontiguousarray(
        np.asarray(weight, dtype=np.float32).reshape(KK, C, NC)
    )
    b = np.ascontiguousarray(np.asarray(bias, dtype=np.float32).reshape(KK, NC))
    with_bias = bool(np.any(b))

    nc = _get_nc(with_bias)
    in_maps = [
        {"x": np.ascontiguousarray(input[i]), "w": w, "b": b} for i in range(B)
    ]
    res = run_bass_kernel_spmd(
        nc, in_maps, core_ids=list(range(B)), trace=_trace
    )
    _cache["last_result"] = res
    out = np.stack(
        [r["out"].reshape(NC, H, W) for r in res.results], axis=0
    )
    return out


# revision 12
# speedup vs baseline: 1.0266x; 1.0266x over previous
"""Trainium2 Bass kernel for nn_CapsuleLayer (capsule conv + 3-iter routing).

Reference (per batch image, C=128, H=W=32, K=3, pad=1):
  priors[h,w,t,nc] = sum_c x_pad[c,h+i,w+j] * W[t,c,nc] + b[t,nc]
  o = mean_t priors
  3x: d2 = sum_cch (o - p_t)^2 ; cw = rsqrt(d2 + 1e-4)
      cw = cw / sum_t cw ; o = sum_t cw_t p_t
  out[nc,h,w] = o

Sharding: data-parallel over batch; 8 cores, one image each; weight/bias
replicated; no collectives.

Implementation notes (m-space routing):
- Work in deviations m_t = p_t - o. Since sum_t alpha_t = 1:
    w = sum_t alpha_t m_t;  o' = o + w;  m'_t = m_t - w
  and d2_t = ||m_t||^2 directly (no <o,p> pass, no ||p||^2 pass, no dist
  assembly). Per iteration the DVE does only: d2 halving-reduce, the
  alpha chain, q = alpha*m, w halving-reduce, m-update.
- The square for d2 runs on the ACT engine (DVE is the bottleneck).
- bf16 on-chip, fp32 PSUM for matmuls; layout [128pos, grp, tap9, cch16,
  cap32] keeps every big DVE op in 2x_1P mode (innermost step-1 cap runs).
- The (cch,cap) permuted channel order is undone for free inside the
  output DMA's address pattern (no un-permute pass).
"""

import numpy as np

C = 128
H = W = 32
B = 8
KK = 9
NCAPS = 32
CCH = 16
NC = NCAPS * CCH  # 512
NIT = 3
NPOS = H * W
CHUNK = 128
GRP = 2  # position-chunks per routing pass
NGRP = NPOS // (CHUNK * GRP)
PADW = 34

_cache = {}


def _build(with_bias: bool):
    import concourse.bass as bass
    import concourse.tile as tile
    from concourse import bacc, mybir
    from concourse.masks import make_identity

    f32 = mybir.dt.float32
    bf16 = mybir.dt.bfloat16
    AF = mybir.ActivationFunctionType

    nc = bacc.Bacc()
    x_d = nc.dram_tensor("x", [C, H, W], f32, kind="ExternalInput")
    w_d = nc.dram_tensor("w", [KK, C, NC], f32, kind="ExternalInput")
    b_d = nc.dram_tensor("b", [KK, NC], f32, kind="ExternalInput")
    out_d = nc.dram_tensor("out", [NC, NPOS], f32, kind="ExternalOutput")

    with tile.TileContext(nc) as tc:
        with (
            tc.tile_pool(name="singles", bufs=1) as singles,
            tc.tile_pool(name="pm", bufs=4) as pm_pool,
            tc.tile_pool(name="big", bufs=1) as big_pool,
            tc.tile_pool(name="half", bufs=1) as half_pool,
            tc.tile_pool(name="wh", bufs=1) as wh_pool,
            tc.tile_pool(name="w", bufs=2) as w_pool,
            tc.tile_pool(name="oa", bufs=2) as oa_pool,
            tc.tile_pool(name="small", bufs=2) as small_pool,
            tc.tile_pool(name="ostage", bufs=1) as ostage_pool,
            tc.tile_pool(name="pp", bufs=4, space="PSUM") as pp,
            tc.tile_pool(name="mp", bufs=2, space="PSUM") as mp,
            tc.tile_pool(name="tpp", bufs=2, space="PSUM") as tpp,
        ):
            # ---- stage inputs: pad + cast to bf16 via gpsimd cast-DMA ----
            xpad = singles.tile([C, PADW * PADW], bf16)
            xpad_v = xpad[:].rearrange("p (h w) -> p h w", h=PADW)
            # zero only the border so the interior cast-DMA needn't wait on
            # a full-tile memset
            nc.gpsimd.memset(xpad_v[:, 0], 0.0)
            nc.gpsimd.memset(xpad_v[:, PADW - 1], 0.0)
            nc.gpsimd.memset(xpad_v[:, 1 : PADW - 1, 0], 0.0)
            nc.gpsimd.memset(xpad_v[:, 1 : PADW - 1, PADW - 1], 0.0)
            nc.gpsimd.dma_start(out=xpad_v[:, 1 : H + 1, 1 : W + 1], in_=x_d[:])

            # column-shifted padded images: xs[j][c, r*32+w] = xpad[c, r, w+j]
            # -> the (i,j)-tap patch for rows h0.. is the CONTIGUOUS slice
            #    xs[j][:, (h0+i)*32 : (h0+i)*32+128]
            xs = []
            for j in range(3):
                xj = singles.tile([C, PADW * W], bf16, tag=f"xs{j}")
                nc.sync.dma_start(
                    out=xj[:].rearrange("p (r w) -> p r w", r=PADW),
                    in_=xpad_v[:, :, j : j + W],
                )
                xs.append(xj)

            # W: natural-order bf16 load (cast in DMA), then per-tap ACT
            # permute (cap,cch)->(cch,cap) so the matmul rhs is contiguous
            wsb = []
            for t in range(KK):
                wt = small_pool.tile([C, NC], bf16, tag="wraw")
                nc.gpsimd.dma_start(out=wt[:], in_=w_d[t])
                wp_t = singles.tile([C, CCH, NCAPS], bf16, tag=f"wsbp{t}")
                nc.scalar.copy(
                    out=wp_t[:],
                    in_=wt[:].rearrange("p (cap cch) -> p cch cap", cch=CCH),
                )
                wsb.append(wp_t)

            ident = singles.tile([128, 128], bf16)
            make_identity(nc, ident[:])

            eps = singles.tile([128, 1], f32)
            nc.gpsimd.memset(eps, 1e-4)

            if with_bias:
                braw = singles.tile([1, KK, NC], bf16)
                nc.gpsimd.dma_start(out=braw[:], in_=b_d[:].unsqueeze(0))
                bsb = singles.tile([1, KK, CCH, NCAPS], bf16)
                nc.scalar.copy(
                    out=bsb[:],
                    in_=braw[:].rearrange("p t (cap cch) -> p t cch cap", cch=CCH),
                )
                ones = singles.tile([1, CHUNK], bf16)
                nc.gpsimd.memset(ones, 1.0)

            for g in range(NGRP):
                # ---- priors + their tap-sum via PE ----
                priors = pm_pool.tile([128, GRP, KK, CCH, NCAPS], bf16, tag="pm")
                OA = oa_pool.tile([128, GRP, NC], bf16)  # running output o
                for cc in range(GRP):
                    ch = GRP * g + cc
                    om = mp.tile([128, NC], f32)  # sum_t priors (fp32)
                    for t in range(KK):
                        i, j = divmod(t, 3)
                        ps = pp.tile([128, NC], f32)
                        lhsT = xs[j][:, 128 * ch + 32 * i : 128 * ch + 32 * i + 128]
                        rhs = wsb[t][:].rearrange("p a b -> p (a b)")
                        if with_bias:
                            nc.tensor.matmul(
                                ps[:], lhsT, rhs, start=True, stop=False
                            )
                            brhs = bsb[:, t].rearrange("p a b -> p (a b)")
                            nc.tensor.matmul(
                                ps[:], ones[:], brhs, start=False, stop=True
                            )
                        else:
                            nc.tensor.matmul(ps[:], lhsT, rhs, start=True, stop=True)
                        nc.tensor.matmul(
                            om[:], lhsT, rhs, start=(t == 0), stop=(t == KK - 1)
                        )
                        if with_bias:
                            nc.tensor.matmul(
                                om[:], ones[:], brhs, start=False, stop=False,
                                skip_group_check=True,
                            )
                        if g == 0 and cc == 0:
                            # DVE is idle during the prologue: help drain the
                            # first chunk's PSUM so priors land sooner
                            nc.vector.tensor_copy(
                                out=priors[:, cc, t],
                                in_=ps[:].rearrange("p (a b) -> p a b", a=CCH),
                            )
                        else:
                            nc.scalar.copy(
                                out=priors[:, cc, t],
                                in_=ps[:].rearrange("p (a b) -> p a b", a=CCH),
                            )
                    # OA = o0 = mean_t priors
                    nc.scalar.activation(
                        out=OA[:, cc], in_=om[:], func=AF.Copy, scale=1.0 / KK
                    )

                # m0 = priors - o0
                m = pm_pool.tile([128, GRP, KK, CCH, NCAPS], bf16, tag="pm")
                ob = (
                    OA[:]
                    .rearrange("p c (a b) -> p c a b", a=CCH)
                    .unsqueeze(2)
                    .broadcast_to((128, GRP, KK, CCH, NCAPS))
                )
                nc.vector.tensor_sub(m[:], priors[:], ob)

                for it in range(NIT):
                    last = it == NIT - 1
                    # d2_t = ||m_t||^2: ACT square + cch halving reduce,
                    # per sub-chunk so DVE overlaps the other half's square
                    sq = big_pool.tile([128, GRP, KK, CCH, NCAPS], bf16, tag="big")
                    h = half_pool.tile([128, GRP, KK, 8, NCAPS], bf16, tag="h")
                    d2 = h[:, :, :, 2]  # dead h slot reused as scratch
                    for cc in range(GRP):
                        nc.scalar.activation(
                            out=sq[:, cc], in_=m[:, cc], func=AF.Square
                        )
                        nc.vector.tensor_add(
                            h[:, cc], sq[:, cc, :, 0:8], sq[:, cc, :, 8:16]
                        )
                        nc.vector.tensor_add(
                            h[:, cc, :, 0:4], h[:, cc, :, 0:4], h[:, cc, :, 4:8]
                        )
                        nc.vector.tensor_add(
                            h[:, cc, :, 0:2], h[:, cc, :, 0:2], h[:, cc, :, 2:4]
                        )
                        nc.vector.tensor_add(
                            d2[:, cc], h[:, cc, :, 0], h[:, cc, :, 1]
                        )

                    # cwu = rsqrt(d2 + eps) on ACT (single table set)
                    cwu = small_pool.tile([128, GRP, KK, NCAPS], bf16, tag="cwu")
                    nc.scalar.activation(
                        out=cwu[:], in_=d2[:], func=AF.Abs_reciprocal_sqrt,
                        bias=eps[:],
                    )
                    # Z = sum_t cwu (halving over taps, final add in fp32);
                    # zh lives in another dead h slot
                    zh = h[:, :, 0:4, 3]
                    nc.vector.tensor_add(
                        zh[:], cwu[:, :, 0:4], cwu[:, :, 4:8]
                    )
                    nc.vector.tensor_add(
                        zh[:, :, 0:2], zh[:, :, 0:2], zh[:, :, 2:4]
                    )
                    nc.vector.tensor_add(zh[:, :, 0], zh[:, :, 0], zh[:, :, 1])
                    z = small_pool.tile([128, GRP, NCAPS], f32, tag="z")
                    nc.vector.tensor_add(z[:], zh[:, :, 0], cwu[:, :, 8])
                    nc.vector.reciprocal_approx_fast(z[:], z[:])
                    rsb = small_pool.tile([128, GRP, NCAPS], bf16, tag="rsb")
                    nc.vector.tensor_scalar_mul(rsb[:], z[:], 1.0)
                    alpha = small_pool.tile([128, GRP, KK, NCAPS], bf16, tag="al")
                    nc.vector.tensor_mul(
                        alpha[:],
                        cwu[:],
                        rsb[:].unsqueeze(2).broadcast_to((128, GRP, KK, NCAPS)),
                    )

                    # q = alpha * m ; w = sum_t q (tap halving)
                    q = big_pool.tile([128, GRP, KK, CCH, NCAPS], bf16, tag="big")
                    ab = alpha[:].unsqueeze(3).broadcast_to(
                        (128, GRP, KK, CCH, NCAPS)
                    )
                    nc.vector.tensor_mul(q[:], m[:], ab)
                    qv = q[:].rearrange("p c t a b -> p c t (a b)")
                    wh = wh_pool.tile([128, GRP, 4, NC], bf16, tag="wh")
                    nc.vector.tensor_add(wh[:], qv[:, :, 0:4], qv[:, :, 4:8])
                    nc.vector.tensor_add(
                        wh[:, :, 0:2], wh[:, :, 0:2], wh[:, :, 2:4]
                    )
                    nc.vector.tensor_add(wh[:, :, 0], wh[:, :, 0], wh[:, :, 1])

                    if not last:
                        w = w_pool.tile([128, GRP, NC], bf16)
                        nc.vector.tensor_add(w[:], wh[:, :, 0], qv[:, :, 8])
                        nc.vector.tensor_add(OA[:], OA[:], w[:])
                        m2 = pm_pool.tile([128, GRP, KK, CCH, NCAPS], bf16, tag="pm")
                        wb = (
                            w[:]
                            .rearrange("p c (a b) -> p c a b", a=CCH)
                            .unsqueeze(2)
                            .broadcast_to((128, GRP, KK, CCH, NCAPS))
                        )
                        nc.vector.tensor_sub(m2[:], m[:], wb)
                        m = m2
                        continue

                    # last iteration: per sub-chunk output assembly so the
                    # transposes overlap the other sub-chunk's tail
                    for cc in range(GRP):
                        ch = GRP * g + cc
                        wfin = w_pool.tile([128, NC], bf16, tag="wfin")
                        nc.vector.tensor_add(
                            wfin[:], wh[:, cc, 0], qv[:, cc, 8]
                        )
                        o3 = w_pool.tile([128, NC], bf16, tag="o3")
                        nc.vector.tensor_add(o3[:], OA[:, cc], wfin[:])
                        ot = ostage_pool.tile([128, 4, 128], f32, tag="ostage")
                        for blk in range(4):
                            tp = tpp.tile([128, 128], bf16)
                            nc.tensor.transpose(
                                tp[:],
                                o3[:, 128 * blk : 128 * (blk + 1)],
                                ident[:],
                            )
                            nc.scalar.copy(out=ot[:, blk], in_=tp[:])
                        # o3 is (cch,cap)-ordered; out_d wants nc=cap*16+cch.
                        # blk holds cch 4*blk..4*blk+3, partition = (c4,cap).
                        nc.sync.dma_start(
                            out=out_d[:, 128 * ch : 128 * (ch + 1)].rearrange(
                                "(cap blk c4) q -> c4 cap blk q", cap=NCAPS,
                                blk=4,
                            ),
                            in_=ot[:],
                        )
    nc.compile()
    return nc


def _get_nc(with_bias: bool):
    key = ("nc", with_bias)
    if key not in _cache:
        _cache[key] = _build(with_bias)
    return _cache[key]


def kernel(input, weight, bias, _trace=False):
    from concourse.bass_utils import run_bass_kernel_spmd

    input = np.ascontiguousarray(np.asarray(input, dtype=np.float32))
    w = np.ascontiguousarray(
        np.asarray(weight, dtype=np.float32).reshape(KK, C, NC)
    )
    b = np.ascontiguousarray(np.asarray(bias, dtype=np.float32).reshape(KK, NC))
    with_bias = bool(np.any(b))

    nc = _get_nc(with_bias)
    in_maps = [
        {"x": np.ascontiguousarray(input[i]), "w": w, "b": b} for i in range(B)
    ]
    res = run_bass_kernel_spmd(
        nc, in_maps, core_ids=list(range(B)), trace=_trace
    )
    _cache["last_result"] = res
    out = np.stack(
        [r["out"].reshape(NC, H, W) for r in res.results], axis=0
    )
    return out


# revision 14
# speedup vs baseline: 1.0690x; 1.0414x over previous
"""Trainium2 Bass kernel for nn_CapsuleLayer (capsule conv + 3-iter routing).

Reference (per batch image, C=128, H=W=32, K=3, pad=1):
  priors[h,w,t,nc] = sum_c x_pad[c,h+i,w+j] * W[t,c,nc] + b[t,nc]
  o = mean_t priors
  3x: d2 = sum_cch (o - p_t)^2 ; cw = rsqrt(d2 + 1e-4)
      cw = cw / sum_t cw ; o = sum_t cw_t p_t
  out[nc,h,w] = o

Sharding: data-parallel over batch; 8 cores, one image each; weight/bias
replicated; no collectives.

Implementation notes (m-space routing):
- Work in deviations m_t = p_t - o. Since sum_t alpha_t = 1:
    w = sum_t alpha_t m_t;  o' = o + w;  m'_t = m_t - w
  and d2_t = ||m_t||^2 directly (no <o,p> pass, no ||p||^2 pass, no dist
  assembly). Per iteration the DVE does only: d2 halving-reduce, the
  alpha chain, q = alpha*m, w halving-reduce, m-update.
- The square for d2 runs on the ACT engine (DVE is the bottleneck).
- bf16 on-chip, fp32 PSUM for matmuls; layout [128pos, grp, tap9, cch16,
  cap32] keeps every big DVE op in 2x_1P mode (innermost step-1 cap runs).
- The (cch,cap) permuted channel order is undone for free inside the
  output DMA's address pattern (no un-permute pass).
"""

import numpy as np

C = 128
H = W = 32
B = 8
KK = 9
NCAPS = 32
CCH = 16
NC = NCAPS * CCH  # 512
NIT = 3
NPOS = H * W
CHUNK = 128
GRP = 2  # position-chunks per routing pass
NGRP = NPOS // (CHUNK * GRP)
PADW = 34

_cache = {}


def _build(with_bias: bool):
    import concourse.bass as bass
    import concourse.tile as tile
    from concourse import bacc, mybir
    from concourse.masks import make_identity

    f32 = mybir.dt.float32
    bf16 = mybir.dt.bfloat16
    AF = mybir.ActivationFunctionType

    nc = bacc.Bacc()
    x_d = nc.dram_tensor("x", [C, H, W], f32, kind="ExternalInput")
    w_d = nc.dram_tensor("w", [KK, C, NC], f32, kind="ExternalInput")
    b_d = nc.dram_tensor("b", [KK, NC], f32, kind="ExternalInput")
    out_d = nc.dram_tensor("out", [NC, NPOS], f32, kind="ExternalOutput")

    with tile.TileContext(nc) as tc:
        with (
            tc.tile_pool(name="singles", bufs=1) as singles,
            tc.tile_pool(name="pm", bufs=3) as pm_pool,
            tc.tile_pool(name="big", bufs=1) as big_pool,
            tc.tile_pool(name="half", bufs=1) as half_pool,
            tc.tile_pool(name="wh", bufs=1) as wh_pool,
            tc.tile_pool(name="w", bufs=2) as w_pool,
            tc.tile_pool(name="oa", bufs=2) as oa_pool,
            tc.tile_pool(name="small", bufs=2) as small_pool,
            tc.tile_pool(name="ostage", bufs=1) as ostage_pool,
            tc.tile_pool(name="pp", bufs=4, space="PSUM") as pp,
            tc.tile_pool(name="mp", bufs=2, space="PSUM") as mp,
            tc.tile_pool(name="tpp", bufs=2, space="PSUM") as tpp,
        ):
            # ---- stage inputs: pad + cast to bf16 via gpsimd cast-DMA ----
            xpad = singles.tile([C, PADW * PADW], bf16)
            xpad_v = xpad[:].rearrange("p (h w) -> p h w", h=PADW)
            # zero only the border so the interior cast-DMA needn't wait on
            # a full-tile memset
            nc.gpsimd.memset(xpad_v[:, 0], 0.0)
            nc.gpsimd.memset(xpad_v[:, PADW - 1], 0.0)
            nc.gpsimd.memset(xpad_v[:, 1 : PADW - 1, 0], 0.0)
            nc.gpsimd.memset(xpad_v[:, 1 : PADW - 1, PADW - 1], 0.0)
            nc.gpsimd.dma_start(out=xpad_v[:, 1 : H + 1, 1 : W + 1], in_=x_d[:])

            # column-shifted padded images: xs[j][c, r*32+w] = xpad[c, r, w+j]
            # -> the (i,j)-tap patch for rows h0.. is the CONTIGUOUS slice
            #    xs[j][:, (h0+i)*32 : (h0+i)*32+128]
            xs = []
            for j in range(3):
                xj = singles.tile([C, PADW * W], bf16, tag=f"xs{j}")
                nc.sync.dma_start(
                    out=xj[:].rearrange("p (r w) -> p r w", r=PADW),
                    in_=xpad_v[:, :, j : j + W],
                )
                xs.append(xj)

            # W: natural-order bf16 load (cast in DMA), then per-tap ACT
            # permute (cap,cch)->(cch,cap) so the matmul rhs is contiguous
            wsb = []
            for t in range(KK):
                wt = small_pool.tile([C, NC], bf16, tag="wraw")
                nc.gpsimd.dma_start(out=wt[:], in_=w_d[t])
                wp_t = singles.tile([C, CCH, NCAPS], bf16, tag=f"wsbp{t}")
                nc.scalar.copy(
                    out=wp_t[:],
                    in_=wt[:].rearrange("p (cap cch) -> p cch cap", cch=CCH),
                )
                wsb.append(wp_t)

            ident = singles.tile([128, 128], bf16)
            make_identity(nc, ident[:])

            eps = singles.tile([128, 1], f32)
            nc.gpsimd.memset(eps, 1e-4)

            if with_bias:
                braw = singles.tile([1, KK, NC], bf16)
                nc.gpsimd.dma_start(out=braw[:], in_=b_d[:].unsqueeze(0))
                bsb = singles.tile([1, KK, CCH, NCAPS], bf16)
                nc.scalar.copy(
                    out=bsb[:],
                    in_=braw[:].rearrange("p t (cap cch) -> p t cch cap", cch=CCH),
                )
                ones = singles.tile([1, CHUNK], bf16)
                nc.gpsimd.memset(ones, 1.0)

            for g in range(NGRP):
                # ---- priors + their tap-sum via PE ----
                priors = pm_pool.tile([128, GRP, KK, CCH, NCAPS], bf16, tag="pm")
                o0 = oa_pool.tile([128, GRP, NC], bf16)
                for cc in range(GRP):
                    ch = GRP * g + cc
                    om = mp.tile([128, NC], f32)  # sum_t priors (fp32)
                    for t in range(KK):
                        i, j = divmod(t, 3)
                        ps = pp.tile([128, NC], f32)
                        lhsT = xs[j][:, 128 * ch + 32 * i : 128 * ch + 32 * i + 128]
                        rhs = wsb[t][:].rearrange("p a b -> p (a b)")
                        if with_bias:
                            nc.tensor.matmul(
                                ps[:], lhsT, rhs, start=True, stop=False
                            )
                            brhs = bsb[:, t].rearrange("p a b -> p (a b)")
                            nc.tensor.matmul(
                                ps[:], ones[:], brhs, start=False, stop=True
                            )
                        else:
                            nc.tensor.matmul(ps[:], lhsT, rhs, start=True, stop=True)
                        nc.tensor.matmul(
                            om[:], lhsT, rhs, start=(t == 0), stop=(t == KK - 1)
                        )
                        if with_bias:
                            nc.tensor.matmul(
                                om[:], ones[:], brhs, start=False, stop=False,
                                skip_group_check=True,
                            )
                        if g == 0 and cc == 0:
                            # DVE is idle during the prologue: help drain the
                            # first chunk's PSUM so priors land sooner
                            nc.vector.tensor_copy(
                                out=priors[:, cc, t],
                                in_=ps[:].rearrange("p (a b) -> p a b", a=CCH),
                            )
                        else:
                            nc.scalar.copy(
                                out=priors[:, cc, t],
                                in_=ps[:].rearrange("p (a b) -> p a b", a=CCH),
                            )
                    nc.scalar.activation(
                        out=o0[:, cc], in_=om[:], func=AF.Copy, scale=1.0 / KK
                    )

                # m0 = priors - o0
                m0 = pm_pool.tile([128, GRP, KK, CCH, NCAPS], bf16, tag="pm")
                ob = (
                    o0[:]
                    .rearrange("p c (a b) -> p c a b", a=CCH)
                    .unsqueeze(2)
                    .broadcast_to((128, GRP, KK, CCH, NCAPS))
                )
                nc.vector.tensor_sub(m0[:], priors[:], ob)

                # it0 distances: d2 = ||m0||^2 (ACT square + cch halving),
                # per sub-chunk so DVE overlaps the other half's square
                sq = big_pool.tile([128, GRP, KK, CCH, NCAPS], bf16, tag="big")
                h = half_pool.tile([128, GRP, KK, 8, NCAPS], bf16, tag="h")
                d2 = small_pool.tile([128, GRP, KK, NCAPS], f32, tag="d2")
                for cc in range(GRP):
                    nc.scalar.activation(
                        out=sq[:, cc], in_=m0[:, cc], func=AF.Square
                    )
                    nc.vector.tensor_add(
                        h[:, cc], sq[:, cc, :, 0:8], sq[:, cc, :, 8:16]
                    )
                    nc.vector.tensor_add(
                        h[:, cc, :, 0:4], h[:, cc, :, 0:4], h[:, cc, :, 4:8]
                    )
                    nc.vector.tensor_add(
                        h[:, cc, :, 0:2], h[:, cc, :, 0:2], h[:, cc, :, 2:4]
                    )
                    nc.vector.tensor_add(
                        d2[:, cc], h[:, cc, :, 0], h[:, cc, :, 1]
                    )

                # fixed-m0 routing: never materialize m1/m2; every product
                # runs against m0.  W_k = sigma_k = sum_t alpha_k,t m0_t;
                # w_k = sigma_k - W_{k-1};
                # d2' = d2 - 2<m_{k-1},w> + ||w||^2
                #     = d2 - ca2 + (e2 + cb2)/2   with ca2 = <m0, 2w>,
                #       e2 = sum_t alpha_t ca2_t, cb2 = <W_{k-1}, 2w>
                Wprev = None
                for it in range(NIT):
                    last = it == NIT - 1
                    # alpha from d2
                    cwu = small_pool.tile([128, GRP, KK, NCAPS], bf16, tag="cwu")
                    nc.scalar.activation(
                        out=cwu[:], in_=d2[:], func=AF.Abs_reciprocal_sqrt,
                        bias=eps[:],
                    )
                    zh = small_pool.tile([128, GRP, 4, NCAPS], bf16, tag="zh")
                    nc.vector.tensor_add(zh[:], cwu[:, :, 0:4], cwu[:, :, 4:8])
                    nc.vector.tensor_add(
                        zh[:, :, 0:2], zh[:, :, 0:2], zh[:, :, 2:4]
                    )
                    nc.vector.tensor_add(zh[:, :, 0], zh[:, :, 0], zh[:, :, 1])
                    z = small_pool.tile([128, GRP, NCAPS], f32, tag="z")
                    nc.vector.tensor_add(z[:], zh[:, :, 0], cwu[:, :, 8])
                    nc.vector.reciprocal_approx_fast(z[:], z[:])
                    rsb = small_pool.tile([128, GRP, NCAPS], bf16, tag="rsb")
                    nc.vector.tensor_scalar_mul(rsb[:], z[:], 1.0)
                    alpha = small_pool.tile([128, GRP, KK, NCAPS], bf16, tag="al")
                    nc.vector.tensor_mul(
                        alpha[:],
                        cwu[:],
                        rsb[:].unsqueeze(2).broadcast_to((128, GRP, KK, NCAPS)),
                    )

                    # sigma = sum_t alpha_t m0_t
                    q = big_pool.tile([128, GRP, KK, CCH, NCAPS], bf16, tag="big")
                    ab = alpha[:].unsqueeze(3).broadcast_to(
                        (128, GRP, KK, CCH, NCAPS)
                    )
                    nc.vector.tensor_mul(q[:], m0[:], ab)
                    qv = q[:].rearrange("p c t a b -> p c t (a b)")
                    wh = wh_pool.tile([128, GRP, 4, NC], bf16, tag="wh")
                    nc.vector.tensor_add(wh[:], qv[:, :, 0:4], qv[:, :, 4:8])
                    nc.vector.tensor_add(
                        wh[:, :, 0:2], wh[:, :, 0:2], wh[:, :, 2:4]
                    )
                    nc.vector.tensor_add(wh[:, :, 0], wh[:, :, 0], wh[:, :, 1])
                    sig = w_pool.tile([128, GRP, NC], bf16, tag="sig")
                    nc.vector.tensor_add(sig[:], wh[:, :, 0], qv[:, :, 8])

                    if last:
                        # o3 = o0 + sigma_3; per sub-chunk output assembly
                        for cc in range(GRP):
                            ch = GRP * g + cc
                            o3 = w_pool.tile([128, NC], bf16, tag="o3")
                            nc.vector.tensor_add(
                                o3[:], o0[:, cc], sig[:, cc]
                            )
                            ot = ostage_pool.tile(
                                [128, 4, 128], f32, tag="ostage"
                            )
                            for blk in range(4):
                                tp = tpp.tile([128, 128], bf16)
                                nc.tensor.transpose(
                                    tp[:],
                                    o3[:, 128 * blk : 128 * (blk + 1)],
                                    ident[:],
                                )
                                nc.scalar.copy(out=ot[:, blk], in_=tp[:])
                            # o3 is (cch,cap)-ordered; out_d wants nc=cap*16+cch
                            nc.sync.dma_start(
                                out=out_d[
                                    :, 128 * ch : 128 * (ch + 1)
                                ].rearrange(
                                    "(cap blk c4) q -> c4 cap blk q",
                                    cap=NCAPS, blk=4,
                                ),
                                in_=ot[:],
                            )
                        continue

                    # w2k = 2*(sigma_k - W_{k-1})
                    if Wprev is None:
                        w2k = w_pool.tile([128, GRP, NC], bf16, tag="wk")
                        nc.vector.tensor_scalar_mul(w2k[:], sig[:], 2.0)
                        cb2 = None
                    else:
                        w2k = w_pool.tile([128, GRP, NC], bf16, tag="wk")
                        nc.vector.tensor_sub(w2k[:], sig[:], Wprev[:])
                        nc.vector.tensor_scalar_mul(w2k[:], w2k[:], 2.0)
                        # cb2 = <W_{k-1}, 2w> (cch halving on 512-dim)
                        cbp = w_pool.tile(
                            [128, GRP, CCH, NCAPS], bf16, tag="cbp"
                        )
                        nc.vector.tensor_mul(
                            cbp[:],
                            Wprev[:].rearrange("p c (a b) -> p c a b", a=CCH),
                            w2k[:].rearrange("p c (a b) -> p c a b", a=CCH),
                        )
                        nc.vector.tensor_add(
                            cbp[:, :, 0:8], cbp[:, :, 0:8], cbp[:, :, 8:16]
                        )
                        nc.vector.tensor_add(
                            cbp[:, :, 0:4], cbp[:, :, 0:4], cbp[:, :, 4:8]
                        )
                        nc.vector.tensor_add(
                            cbp[:, :, 0:2], cbp[:, :, 0:2], cbp[:, :, 2:4]
                        )
                        cb2 = small_pool.tile([128, GRP, NCAPS], f32, tag="cb")
                        nc.vector.tensor_add(
                            cb2[:], cbp[:, :, 0], cbp[:, :, 1]
                        )
                    Wprev = sig

                    # ca2 = <m0_t, 2w> per tap/cap
                    ca = big_pool.tile(
                        [128, GRP, KK, CCH, NCAPS], bf16, tag="big"
                    )
                    wb = (
                        w2k[:]
                        .rearrange("p c (a b) -> p c a b", a=CCH)
                        .unsqueeze(2)
                        .broadcast_to((128, GRP, KK, CCH, NCAPS))
                    )
                    nc.vector.tensor_mul(ca[:], m0[:], wb)
                    ch_ = half_pool.tile([128, GRP, KK, 8, NCAPS], bf16, tag="h")
                    nc.vector.tensor_add(
                        ch_[:], ca[:, :, :, 0:8], ca[:, :, :, 8:16]
                    )
                    nc.vector.tensor_add(
                        ch_[:, :, :, 0:4], ch_[:, :, :, 0:4], ch_[:, :, :, 4:8]
                    )
                    nc.vector.tensor_add(
                        ch_[:, :, :, 0:2], ch_[:, :, :, 0:2], ch_[:, :, :, 2:4]
                    )
                    ca2 = small_pool.tile([128, GRP, KK, NCAPS], bf16, tag="c2")
                    nc.vector.tensor_add(
                        ca2[:], ch_[:, :, :, 0], ch_[:, :, :, 1]
                    )

                    # e2 = sum_t alpha_t ca2_t ; u = (e2 + cb2) / 2
                    tm = small_pool.tile([128, GRP, KK, NCAPS], bf16, tag="tm")
                    nc.vector.tensor_mul(tm[:], alpha[:], ca2[:])
                    eh = small_pool.tile([128, GRP, 4, NCAPS], bf16, tag="eh")
                    nc.vector.tensor_add(eh[:], tm[:, :, 0:4], tm[:, :, 4:8])
                    nc.vector.tensor_add(
                        eh[:, :, 0:2], eh[:, :, 0:2], eh[:, :, 2:4]
                    )
                    nc.vector.tensor_add(eh[:, :, 0], eh[:, :, 0], eh[:, :, 1])
                    u = small_pool.tile([128, GRP, NCAPS], f32, tag="u")
                    nc.vector.tensor_add(u[:], eh[:, :, 0], tm[:, :, 8])
                    if cb2 is not None:
                        nc.vector.tensor_add(u[:], u[:], cb2[:])
                    nc.vector.tensor_scalar_mul(u[:], u[:], 0.5)

                    # d2' = (d2 - ca2) + u
                    d2n = small_pool.tile([128, GRP, KK, NCAPS], f32, tag="d2")
                    nc.vector.tensor_sub(d2n[:], d2[:], ca2[:])
                    nc.vector.tensor_add(
                        d2n[:],
                        d2n[:],
                        u[:].unsqueeze(2).broadcast_to((128, GRP, KK, NCAPS)),
                    )
                    d2 = d2n
    nc.compile()
    return nc


def _get_nc(with_bias: bool):
    key = ("nc", with_bias)
    if key not in _cache:
        _cache[key] = _build(with_bias)
    return _cache[key]


def kernel(input, weight, bias, _trace=False):
    from concourse.bass_utils import run_bass_kernel_spmd

    input = np.ascontiguousarray(np.asarray(input, dtype=np.float32))
    w = np.ascontiguousarray(
        np.asarray(weight, dtype=np.float32).reshape(KK, C, NC)
    )
    b = np.ascontiguousarray(np.asarray(bias, dtype=np.float32).reshape(KK, NC))
    with_bias = bool(np.any(b))

    nc = _get_nc(with_bias)
    in_maps = [
        {"x": np.ascontiguousarray(input[i]), "w": w, "b": b} for i in range(B)
    ]
    res = run_bass_kernel_spmd(
        nc, in_maps, core_ids=list(range(B)), trace=_trace
    )
    _cache["last_result"] = res
    out = np.stack(
        [r["out"].reshape(NC, H, W) for r in res.results], axis=0
    )
    return out
